# revision 34
# baseline (speedup 1.0000x reference)
"""LocallyConnected2d (3x3, pad 1) Trainium2 kernel.

Problem: out[b,o,h,w] = sum_{c,k} x_pad[b,c,h+k//3,w+k%3] * W[o,c,h,w,k]
  x: [16, 64, 56, 56] f32, W: [1, 64, 64, 56, 56, 9] f32 -> out [16, 64, 56, 56] f32

Strategy (8 cores, H sharded, 7 output rows per core):
  The weight (462 MB) is used exactly once -> the kernel is bound by
  streaming it HBM->SBUF->PE. W is quantized to fp8 e3m4 (x32 scale; 4
  mantissa bits -> rel err ~1.3e-2 vs the 2e-2 gate) and x to fp16 (/32, so
  the product is exact); the PE runs the mixed fp16-stationary x fp8-moving
  matmul natively, and PSUM accumulates in f32.

  PE shape: 8 output locations are packed into every matmul so the moving
  operand is wide (N=512) and the per-matmul fixed costs amortize:
    stationary = x patches [K, 8 locs x 16 batch = 128 cols]
    moving     = 8 locations' weight slices [K, 8 x 64 outch = 512 cols]
    psum       = [128, 512] f32 = exactly one 2 KB PSUM bank
  Only the 8 diagonal [16, 64] blocks of each product are real output (7/8
  of the MACs are discarded -- PE time is N cycles, so this is free).
  Engines require 32-aligned partition bases, so the drain copies pull
  32-partition x 128-col blocks (the real [16, 64] block plus its
  neighbor's garbage) as fp16, alternating DVE/ACT; the host drops the
  garbage halves.

  Per location the 9 taps x 64 channels contract as 5 accumulating matmuls:
    - chunks 0-2 (K=128): tap pairs {0,3},{1,4},{2,5} stacked on the
      partition dim; the upper 64 partitions hold x shifted by one padded
      row (58) so one AP reads both taps.
    - chunk 3 (K=128): taps {6,7}, upper half shifted by 1.
    - chunk 4 (K=64): tap 8 on one partition half; even rows use the lower
      half (plain x), odd rows the upper half (shift-1 x) with the slot's
      other half carrying the adjacent row's tap-8 weights, so slots stay
      dense. Row 6 (unpaired) wastes its upper halves.
  x is staged as [128, region, f, b] (batch innermost) so the 128 stationary
  columns of each matmul are one strided AP.
"""

import numpy as np

B, C, O, H, W = 16, 64, 64, 56, 56
NCORES = 8
HPC = H // NCORES          # 7 output rows per core
XROWS = HPC + 2            # 9 padded-x rows per core
XW = W + 2                 # 58
BLK = XROWS * XW           # 522 x columns per (c)
XTRIM = BLK - 2 * XW       # 406 cols actually addressed per region
XOFF = 2 * XW              # region-0 window starts at padded row 2
GRP = 8                    # output locations per matmul
NG = W // GRP              # 7 groups per row
# weight slots: [g][c0 x8][c1 x8][c2 x8][c3 x8][(c4 x8 if h even)]
NS_EVEN = NG * 5 * GRP     # 280 slots (rows 0,2,4,6: carry tap8 for h,h+1)
NS_ODD = NG * 4 * GRP      # 224 slots
SLAB_OFF = [0, 280, 504, 784, 1008, 1288, 1512]
NS_TOT = 1792
WSCALE = 32.0              # w stored as e3m4*32, x as fp16/32

_CACHE = {}


def _host_prep(x, weight):
    """Build per-core device input arrays (layout transforms, host-side only)."""
    x = np.ascontiguousarray(x, dtype=np.float32)
    w0 = weight.reshape(O, C, H, W, 9).astype(np.float32, copy=False)

    xpad = np.zeros((B, C, H + 2, W + 2), np.float32)
    xpad[:, :, 1:-1, 1:-1] = x

    xs_list, ws_list = [], []
    for core in range(NCORES):
        h0 = core * HPC
        # x: [128, region, f, b]; region 0 = [plain; shift-1],
        # region 1 = [plain; shift-58]; b innermost.
        xc = xpad[:, :, h0:h0 + XROWS, :]                     # [B, C, 9, 58]
        plain = np.ascontiguousarray(
            xc.transpose(1, 2, 3, 0)).reshape(C, BLK, B)      # [C, f, b]
        sh1 = np.zeros_like(plain)
        sh1[:, :-1] = plain[:, 1:]
        sh58 = np.zeros_like(plain)
        sh58[:, :-58] = plain[:, 58:]
        xdev = np.empty((128, 2, XTRIM, B), np.float32)
        xdev[:64, 0] = plain[:, XOFF:]
        xdev[64:, 0] = sh1[:, XOFF:]
        xdev[:64, 1] = plain[:, :XTRIM]
        xdev[64:, 1] = sh58[:, :XTRIM]
        xs_list.append((xdev / WSCALE).astype(np.float16))

        # weight slots: [128, NS_TOT, 64]
        wc = w0[:, :, h0:h0 + HPC, :, :]                      # [O, C, 7, 56, 9]
        wt = np.ascontiguousarray(wc.transpose(1, 4, 2, 3, 0))  # [C, 9, 7, 56, O]
        S = np.zeros((128, NS_TOT, O), np.float32)
        for h in range(HPC):
            off = SLAB_OFF[h]
            bs = 5 * GRP if h % 2 == 0 else 4 * GRP
            for g in range(NG):
                base = off + g * bs
                w_lo = g * GRP
                for t in range(3):
                    S[:64, base + t * GRP: base + (t + 1) * GRP] = \
                        wt[:, t, h, w_lo:w_lo + GRP]
                    S[64:, base + t * GRP: base + (t + 1) * GRP] = \
                        wt[:, t + 3, h, w_lo:w_lo + GRP]
                S[:64, base + 24: base + 32] = wt[:, 6, h, w_lo:w_lo + GRP]
                S[64:, base + 24: base + 32] = wt[:, 7, h, w_lo:w_lo + GRP]
                if h % 2 == 0:
                    S[:64, base + 32: base + 40] = wt[:, 8, h, w_lo:w_lo + GRP]
                    if h + 1 < HPC:
                        S[64:, base + 32: base + 40] = \
                            wt[:, 8, h + 1, w_lo:w_lo + GRP]
        import ml_dtypes
        ws_list.append((S * WSCALE).astype(ml_dtypes.float8_e3m4))
    return xs_list, ws_list


def _build_program(repeat=1, mode="full"):
    import concourse.mybir as mybir
    import concourse.tile as tile
    from concourse import bacc

    f32 = mybir.dt.float32
    f16 = mybir.dt.float16
    f8 = mybir.dt.float8e3
    nc = bacc.Bacc("TRN2", target_bir_lowering=False, debug=False,
                   num_devices=NCORES)
    xs = nc.dram_tensor("xs", [128, 2, XTRIM, B], f16,
                        kind="ExternalInput")
    ws = nc.dram_tensor("ws", [128, NS_TOT, O], f8, kind="ExternalInput")
    out = nc.dram_tensor("out", [HPC, 128, NG, 2, O], f16,
                         kind="ExternalOutput")

    with tile.TileContext(nc) as tc:
        with tc.tile_pool(name="xp", bufs=2) as xpool, \
             tc.tile_pool(name="xq", bufs=1) as xqpool, \
             tc.tile_pool(name="wp", bufs=4) as wpool, \
             tc.tile_pool(name="op", bufs=3) as opool, \
             tc.tile_pool(name="pq", bufs=3, space="PSUM") as pqpool, \
             tc.tile_pool(name="pp", bufs=1, space="PSUM") as ppool:

            xt_pre = None
            if mode.endswith("noxs") or mode == "pe_xs2":
                xt_pre = xqpool.tile([128, 2, XTRIM, B], f16, name="xtp")
                nc.scalar.dma_start(xt_pre[:], xs[:])
            wt_pre = None
            if mode == "full_wthru":
                wt_pre = xqpool.tile([128, NS_EVEN, O], f8, name="wtp")
                nc.sync.dma_start(wt_pre[:], ws[:, 0:NS_EVEN, :])

            def body(_iv=None):
                # Queue split: weight slabs own the SP queue (pure stream);
                # xs rides ACT, out rides GPSIMD -- a DMA start that waits on
                # compute must not sit ahead of weight prefetches in the
                # strict-FIFO SP queue.
                if xt_pre is not None:
                    xt = xt_pre
                    if mode == "pe_xs2":
                        jt = xpool.tile([128, 2, XTRIM, B], f16, name="xt")
                        nc.scalar.dma_start(jt[:, :, 0:3 * XW, :],
                                            xs[:, :, 0:3 * XW, :])
                        nc.scalar.dma_start(jt[:, :, 3 * XW:, :],
                                            xs[:, :, 3 * XW:, :])
                else:
                    xt = xpool.tile([128, 2, XTRIM, B], f16, name="xt")
                    nc.scalar.dma_start(xt[:, :, 0:3 * XW, :],
                                        xs[:, :, 0:3 * XW, :])
                    nc.scalar.dma_start(xt[:, :, 3 * XW:, :],
                                        xs[:, :, 3 * XW:, :])
                wt0 = None
                if mode.startswith("pe"):
                    wt0 = wpool.tile([128, NS_EVEN, O], f8, name="wt")
                    nc.sync.dma_start(wt0[:], ws[:, 0:NS_EVEN, :])
                ncopy = 0
                wt_prev = None
                for h in range(HPC):
                    ns = NS_EVEN if h % 2 == 0 else NS_ODD
                    bs = 5 * GRP if h % 2 == 0 else 4 * GRP
                    if mode.startswith("pe"):
                        wt = wt0
                    else:
                        wt = wpool.tile([128, ns, O], f8, name="wt")
                        o0 = SLAB_OFF[h]
                        nc.sync.dma_start(
                            wt[:, 0:ns // 2, :], ws[:, o0:o0 + ns // 2, :])
                        nc.sync.dma_start(
                            wt[:, ns // 2:, :], ws[:, o0 + ns // 2:o0 + ns, :])
                        if wt_pre is not None:
                            wt = wt_pre
                    if mode == "dma":
                        wt_prev = wt
                        continue
                    ot = opool.tile([128, NG, 2, O], f16, name="ot")
                    pq = None
                    for g in range(NG):
                        base = g * bs
                        F = h * XW + g * GRP
                        G = h * XW + g * GRP
                        if g % 2 == 0:
                            pq = pqpool.tile([128, 2, GRP, O], f32,
                                             name="pq")
                        ps = pq[:, g % 2]
                        for t in range(3):
                            nc.tensor.matmul(
                                ps[:], xt[:, 1, F + t:F + t + GRP, :],
                                wt[:, base + t * GRP:base + (t + 1) * GRP, :],
                                start=(t == 0), stop=False)
                        nc.tensor.matmul(
                            ps[:], xt[:, 0, G:G + GRP, :],
                            wt[:, base + 24:base + 32, :],
                            start=False, stop=(mode == "pe4"))
                        if mode == "pe4":
                            continue
                        if h % 2 == 0 or mode.startswith("pe"):
                            c4 = wt if not mode.startswith("pe") else wt0
                            nc.tensor.matmul(
                                ps[:], xt[0:64, 0, G + 2:G + 2 + GRP, :],
                                c4[0:64, base + 32:base + 40, :],
                                start=False, stop=True)
                        else:
                            pbase = g * 5 * GRP
                            nc.tensor.matmul(
                                ps[:], xt[64:128, 0, G + 1:G + 1 + GRP, :],
                                wt_prev[64:128, pbase + 32:pbase + 40, :],
                                start=False, stop=True)
                        if mode in ("pe_nodrain", "pe_noxs", "pe_xs2"):
                            continue
                        if g % 2 == 1 or g == NG - 1:
                            g0, np_ = (g - 1, 2) if g % 2 == 1 else (g, 1)
                            for j in range(GRP // 2):
                                dst = ot[32 * j:32 * (j + 1),
                                         g0:g0 + np_, :, :]
                                src = pq[32 * j:32 * (j + 1), 0:np_,
                                         2 * j:2 * j + 2, :]
                                if ncopy % 2 == 0:
                                    nc.vector.tensor_copy(dst, src)
                                else:
                                    nc.scalar.copy(dst, src)
                                ncopy += 1
                    if mode in ("pe_nodrain", "pe4", "pe_noxs", "pe_xs2"):
                        continue
                    if h == HPC - 1:
                        # split the last row's out-DMA so the first half
                        # fires before the final drain copies (shorter tail)
                        nc.gpsimd.dma_start(out[h, 0:64], ot[0:64])
                        nc.gpsimd.dma_start(out[h, 64:128], ot[64:128])
                    else:
                        nc.gpsimd.dma_start(out[h], ot[:])
                    wt_prev = wt

            if repeat > 1:
                # hardware loop re-executes the same instructions, so tiles
                # can't rotate across iterations; unroll 2 bodies per trip so
                # the next body's xs DMA overlaps the current body's compute.
                assert repeat % 2 == 0
                with tc.For_i(0, repeat // 2, 1):
                    body()
                    body()
            else:
                body()
    nc.compile()
    return nc


def _get_program(repeat=1, mode="full"):
    key = ("nc", repeat, mode)
    if key not in _CACHE:
        _CACHE[key] = _build_program(repeat, mode)
    return _CACHE[key]


def run(x, weight, trace=False, repeat=1, mode="full"):
    from concourse.bass_utils import run_bass_kernel_spmd

    nc = _get_program(repeat, mode)
    xs_list, ws_list = _host_prep(np.asarray(x), np.asarray(weight))
    in_maps = [{"xs": xs_list[i], "ws": ws_list[i]} for i in range(NCORES)]
    res = run_bass_kernel_spmd(nc, in_maps, core_ids=list(range(NCORES)),
                               trace=trace)
    parts = []
    for i in range(NCORES):
        oc = np.asarray(res.results[i]["out"])       # [HPC, 128, NG, 2, O]
        oc = oc.reshape(HPC, 4, 2, B, NG, 2, O)      # [h, j, i2, b, g, e, o]
        real = np.stack([oc[:, :, 0, :, :, 0], oc[:, :, 1, :, :, 1]],
                        axis=2)                      # [h, j, i2, b, g, o]
        parts.append(real.transpose(3, 5, 0, 4, 1, 2)
                     .astype(np.float32).reshape(B, O, HPC, W))
    full = np.concatenate(parts, axis=2)             # [B, O, H, W]
    return np.ascontiguousarray(full), res


def kernel(x, weight):
    out, _ = run(x, weight, trace=False)
    return out



# revision 36
# speedup vs baseline: 1.0544x; 1.0544x over previous
"""LocallyConnected2d (3x3, pad 1) Trainium2 kernel.

Problem: out[b,o,h,w] = sum_{c,k} x_pad[b,c,h+k//3,w+k%3] * W[o,c,h,w,k]
  x: [16, 64, 56, 56] f32, W: [1, 64, 64, 56, 56, 9] f32 -> out [16, 64, 56, 56] f32

Strategy (8 cores, H sharded, 7 output rows per core):
  The weight (462 MB) is used exactly once -> the kernel is bound by
  streaming it HBM->SBUF->PE. W is quantized to fp8 e3m4 (x32 scale; 4
  mantissa bits -> rel err ~1.3e-2 vs the 2e-2 gate) and x to fp16 (/32, so
  the product is exact); the PE runs the mixed fp16-stationary x fp8-moving
  matmul natively, and PSUM accumulates in f32.

  PE shape: 8 output locations are packed into every matmul so the moving
  operand is wide (N=512) and the per-matmul fixed costs amortize:
    stationary = x patches [K, 8 locs x 16 batch = 128 cols]
    moving     = 8 locations' weight slices [K, 8 x 64 outch = 512 cols]
    psum       = [128, 512] f32 = exactly one 2 KB PSUM bank
  Only the 8 diagonal [16, 64] blocks of each product are real output (7/8
  of the MACs are discarded -- PE time is N cycles, so this is free).
  Engines require 32-aligned partition bases, so the drain copies pull
  32-partition x 128-col blocks (the real [16, 64] block plus its
  neighbor's garbage) as fp16, alternating DVE/ACT; the host drops the
  garbage halves.

  Per location the 9 taps x 64 channels contract as 5 accumulating matmuls:
    - chunks 0-2 (K=128): tap pairs {0,3},{1,4},{2,5} stacked on the
      partition dim; the upper 64 partitions hold x shifted by one padded
      row (58) so one AP reads both taps.
    - chunk 3 (K=128): taps {6,7}, upper half shifted by 1.
    - chunk 4 (K=64): tap 8 on one partition half; even rows use the lower
      half (plain x), odd rows the upper half (shift-1 x) with the slot's
      other half carrying the adjacent row's tap-8 weights, so slots stay
      dense. Row 6 (unpaired) wastes its upper halves.
  x is staged as [128, region, f, b] (batch innermost) so the 128 stationary
  columns of each matmul are one strided AP.
"""

import numpy as np

B, C, O, H, W = 16, 64, 64, 56, 56
NCORES = 8
HPC = H // NCORES          # 7 output rows per core
XROWS = HPC + 2            # 9 padded-x rows per core
XW = W + 2                 # 58
BLK = XROWS * XW           # 522 x columns per (c)
XTRIM = BLK - 2 * XW       # 406 cols actually addressed per region
XOFF = 2 * XW              # region-0 window starts at padded row 2
GRP = 8                    # output locations per matmul
NG = W // GRP              # 7 groups per row
# weight slots: [g][c0 x8][c1 x8][c2 x8][c3 x8][(c4 x8 if h even)]
NS_EVEN = NG * 5 * GRP     # 280 slots (rows 0,2,4,6: carry tap8 for h,h+1)
NS_ODD = NG * 4 * GRP      # 224 slots
SLAB_OFF = [0, 280, 504, 784, 1008, 1288, 1512]
NS_TOT = 1792
WSCALE = 32.0              # w stored as e3m4*32, x as fp16/32

_CACHE = {}


def _host_prep(x, weight):
    """Build per-core device input arrays (layout transforms, host-side only)."""
    x = np.ascontiguousarray(x, dtype=np.float32)
    w0 = weight.reshape(O, C, H, W, 9).astype(np.float32, copy=False)

    xpad = np.zeros((B, C, H + 2, W + 2), np.float32)
    xpad[:, :, 1:-1, 1:-1] = x

    xs_list, ws_list = [], []
    for core in range(NCORES):
        h0 = core * HPC
        # x: [128, region, f, b]; region 0 = [plain; shift-1],
        # region 1 = [plain; shift-58]; b innermost.
        xc = xpad[:, :, h0:h0 + XROWS, :]                     # [B, C, 9, 58]
        plain = np.ascontiguousarray(
            xc.transpose(1, 2, 3, 0)).reshape(C, BLK, B)      # [C, f, b]
        sh1 = np.zeros_like(plain)
        sh1[:, :-1] = plain[:, 1:]
        sh58 = np.zeros_like(plain)
        sh58[:, :-58] = plain[:, 58:]
        xdev = np.empty((128, 2, XTRIM, B), np.float32)
        xdev[:64, 0] = plain[:, XOFF:]
        xdev[64:, 0] = sh1[:, XOFF:]
        xdev[:64, 1] = plain[:, :XTRIM]
        xdev[64:, 1] = sh58[:, :XTRIM]
        xs_list.append((xdev / WSCALE).astype(np.float16))

        # weight slots: [128, NS_TOT, 64]
        wc = w0[:, :, h0:h0 + HPC, :, :]                      # [O, C, 7, 56, 9]
        wt = np.ascontiguousarray(wc.transpose(1, 4, 2, 3, 0))  # [C, 9, 7, 56, O]
        S = np.zeros((128, NS_TOT, O), np.float32)
        for h in range(HPC):
            off = SLAB_OFF[h]
            bs = 5 * GRP if h % 2 == 0 else 4 * GRP
            for g in range(NG):
                base = off + g * bs
                w_lo = g * GRP
                for t in range(3):
                    S[:64, base + t * GRP: base + (t + 1) * GRP] = \
                        wt[:, t, h, w_lo:w_lo + GRP]
                    S[64:, base + t * GRP: base + (t + 1) * GRP] = \
                        wt[:, t + 3, h, w_lo:w_lo + GRP]
                S[:64, base + 24: base + 32] = wt[:, 6, h, w_lo:w_lo + GRP]
                S[64:, base + 24: base + 32] = wt[:, 7, h, w_lo:w_lo + GRP]
                if h % 2 == 0:
                    S[:64, base + 32: base + 40] = wt[:, 8, h, w_lo:w_lo + GRP]
                    if h + 1 < HPC:
                        S[64:, base + 32: base + 40] = \
                            wt[:, 8, h + 1, w_lo:w_lo + GRP]
        import ml_dtypes
        ws_list.append((S * WSCALE).astype(ml_dtypes.float8_e3m4))
    return xs_list, ws_list


def _build_program(repeat=1, mode="full"):
    import concourse.mybir as mybir
    import concourse.tile as tile
    from concourse import bacc

    f32 = mybir.dt.float32
    f16 = mybir.dt.float16
    f8 = mybir.dt.float8e3
    nc = bacc.Bacc("TRN2", target_bir_lowering=False, debug=False,
                   num_devices=NCORES)
    xs = nc.dram_tensor("xs", [128, 2, XTRIM, B], f16,
                        kind="ExternalInput")
    ws = nc.dram_tensor("ws", [128, NS_TOT, O], f8, kind="ExternalInput")
    out = nc.dram_tensor("out", [HPC, 128, NG, 2, O], f16,
                         kind="ExternalOutput")

    with tile.TileContext(nc) as tc:
        with tc.tile_pool(name="xp",
                          bufs=(2 if mode in ("full_u2", "full_2dma")
                                else 4)) as xpool, \
             tc.tile_pool(name="xq", bufs=1) as xqpool, \
             tc.tile_pool(name="wp", bufs=4) as wpool, \
             tc.tile_pool(name="op", bufs=3) as opool, \
             tc.tile_pool(name="pq", bufs=3, space="PSUM") as pqpool, \
             tc.tile_pool(name="pp", bufs=1, space="PSUM") as ppool:

            xt_pre = None
            if mode.endswith("noxs") or mode == "pe_xs2":
                xt_pre = xqpool.tile([128, 2, XTRIM, B], f16, name="xtp")
                nc.scalar.dma_start(xt_pre[:], xs[:])
            wt_pre = None
            if mode == "full_wthru":
                wt_pre = xqpool.tile([128, NS_EVEN, O], f8, name="wtp")
                nc.sync.dma_start(wt_pre[:], ws[:, 0:NS_EVEN, :])

            def body(_iv=None):
                # Queue split: weight slabs own the SP queue (pure stream);
                # xs rides ACT, out rides GPSIMD -- a DMA start that waits on
                # compute must not sit ahead of weight prefetches in the
                # strict-FIFO SP queue.
                if xt_pre is not None:
                    xt = xt_pre
                    if mode == "pe_xs2":
                        jt = xpool.tile([128, 2, XTRIM, B], f16, name="xt")
                        nc.scalar.dma_start(jt[:, :, 0:3 * XW, :],
                                            xs[:, :, 0:3 * XW, :])
                        nc.scalar.dma_start(jt[:, :, 3 * XW:, :],
                                            xs[:, :, 3 * XW:, :])
                else:
                    xt = xpool.tile([128, 2, XTRIM, B], f16, name="xt")
                    if mode != "full_2dma":
                        for c0, c1 in ((0, 102), (102, 204), (204, 306),
                                       (306, XTRIM)):
                            nc.scalar.dma_start(xt[:, :, c0:c1, :],
                                                xs[:, :, c0:c1, :])
                    else:
                        nc.scalar.dma_start(xt[:, :, 0:3 * XW, :],
                                            xs[:, :, 0:3 * XW, :])
                        nc.scalar.dma_start(xt[:, :, 3 * XW:, :],
                                            xs[:, :, 3 * XW:, :])
                wt0 = None
                if mode.startswith("pe"):
                    wt0 = wpool.tile([128, NS_EVEN, O], f8, name="wt")
                    nc.sync.dma_start(wt0[:], ws[:, 0:NS_EVEN, :])
                ncopy = 0
                wt_prev = None
                for h in range(HPC):
                    ns = NS_EVEN if h % 2 == 0 else NS_ODD
                    bs = 5 * GRP if h % 2 == 0 else 4 * GRP
                    if mode.startswith("pe"):
                        wt = wt0
                    else:
                        wt = wpool.tile([128, ns, O], f8, name="wt")
                        o0 = SLAB_OFF[h]
                        nc.sync.dma_start(
                            wt[:, 0:ns // 2, :], ws[:, o0:o0 + ns // 2, :])
                        nc.sync.dma_start(
                            wt[:, ns // 2:, :], ws[:, o0 + ns // 2:o0 + ns, :])
                        if wt_pre is not None:
                            wt = wt_pre
                    if mode == "dma":
                        wt_prev = wt
                        continue
                    ot = opool.tile([128, NG, 2, O], f16, name="ot")
                    pq = None
                    for g in range(NG):
                        base = g * bs
                        F = h * XW + g * GRP
                        G = h * XW + g * GRP
                        if g % 2 == 0:
                            pq = pqpool.tile([128, 2, GRP, O], f32,
                                             name="pq")
                        ps = pq[:, g % 2]
                        for t in range(3):
                            nc.tensor.matmul(
                                ps[:], xt[:, 1, F + t:F + t + GRP, :],
                                wt[:, base + t * GRP:base + (t + 1) * GRP, :],
                                start=(t == 0), stop=False)
                        nc.tensor.matmul(
                            ps[:], xt[:, 0, G:G + GRP, :],
                            wt[:, base + 24:base + 32, :],
                            start=False, stop=(mode == "pe4"))
                        if mode == "pe4":
                            continue
                        if h % 2 == 0 or mode.startswith("pe"):
                            c4 = wt if not mode.startswith("pe") else wt0
                            nc.tensor.matmul(
                                ps[:], xt[0:64, 0, G + 2:G + 2 + GRP, :],
                                c4[0:64, base + 32:base + 40, :],
                                start=False, stop=True)
                        else:
                            pbase = g * 5 * GRP
                            nc.tensor.matmul(
                                ps[:], xt[64:128, 0, G + 1:G + 1 + GRP, :],
                                wt_prev[64:128, pbase + 32:pbase + 40, :],
                                start=False, stop=True)
                        if mode in ("pe_nodrain", "pe_noxs", "pe_xs2"):
                            continue
                        if g % 2 == 1 or g == NG - 1:
                            g0, np_ = (g - 1, 2) if g % 2 == 1 else (g, 1)
                            for j in range(GRP // 2):
                                dst = ot[32 * j:32 * (j + 1),
                                         g0:g0 + np_, :, :]
                                src = pq[32 * j:32 * (j + 1), 0:np_,
                                         2 * j:2 * j + 2, :]
                                if ncopy % 2 == 0:
                                    nc.vector.tensor_copy(dst, src)
                                else:
                                    nc.scalar.copy(dst, src)
                                ncopy += 1
                    if mode in ("pe_nodrain", "pe4", "pe_noxs", "pe_xs2"):
                        continue
                    if h == HPC - 1:
                        # split the last row's out-DMA so the first half
                        # fires before the final drain copies (shorter tail)
                        nc.gpsimd.dma_start(out[h, 0:64], ot[0:64])
                        nc.gpsimd.dma_start(out[h, 64:128], ot[64:128])
                    else:
                        nc.gpsimd.dma_start(out[h], ot[:])
                    wt_prev = wt

            if repeat > 1:
                # hardware loop re-executes the same instructions, so tiles
                # can't rotate across iterations; unroll 2 bodies per trip so
                # the next body's xs DMA overlaps the current body's compute.
                nun = 2 if mode in ("full_u2", "full_2dma") else 4
                assert repeat % nun == 0
                with tc.For_i(0, repeat // nun, 1):
                    for _ in range(nun):
                        body()
            else:
                body()
    nc.compile()
    return nc


def _get_program(repeat=1, mode="full"):
    key = ("nc", repeat, mode)
    if key not in _CACHE:
        _CACHE[key] = _build_program(repeat, mode)
    return _CACHE[key]


def run(x, weight, trace=False, repeat=1, mode="full"):
    from concourse.bass_utils import run_bass_kernel_spmd

    nc = _get_program(repeat, mode)
    xs_list, ws_list = _host_prep(np.asarray(x), np.asarray(weight))
    in_maps = [{"xs": xs_list[i], "ws": ws_list[i]} for i in range(NCORES)]
    res = run_bass_kernel_spmd(nc, in_maps, core_ids=list(range(NCORES)),
                               trace=trace)
    parts = []
    for i in range(NCORES):
        oc = np.asarray(res.results[i]["out"])       # [HPC, 128, NG, 2, O]
        oc = oc.reshape(HPC, 4, 2, B, NG, 2, O)      # [h, j, i2, b, g, e, o]
        real = np.stack([oc[:, :, 0, :, :, 0], oc[:, :, 1, :, :, 1]],
                        axis=2)                      # [h, j, i2, b, g, o]
        parts.append(real.transpose(3, 5, 0, 4, 1, 2)
                     .astype(np.float32).reshape(B, O, HPC, W))
    full = np.concatenate(parts, axis=2)             # [B, O, H, W]
    return np.ascontiguousarray(full), res


def kernel(x, weight):
    out, _ = run(x, weight, trace=False)
    return out



# revision 38
# speedup vs baseline: 1.0958x; 1.0392x over previous
"""LocallyConnected2d (3x3, pad 1) Trainium2 kernel.

Problem: out[b,o,h,w] = sum_{c,k} x_pad[b,c,h+k//3,w+k%3] * W[o,c,h,w,k]
  x: [16, 64, 56, 56] f32, W: [1, 64, 64, 56, 56, 9] f32 -> out [16, 64, 56, 56] f32

Strategy (8 cores, H sharded, 7 output rows per core):
  The weight (462 MB) is used exactly once -> the kernel is bound by
  streaming it HBM->SBUF->PE. W is quantized to fp8 e3m4 (x32 scale; 4
  mantissa bits -> rel err ~1.3e-2 vs the 2e-2 gate) and x to fp16 (/32, so
  the product is exact); the PE runs the mixed fp16-stationary x fp8-moving
  matmul natively, and PSUM accumulates in f32.

  PE shape: 8 output locations are packed into every matmul so the moving
  operand is wide (N=512) and the per-matmul fixed costs amortize:
    stationary = x patches [K, 8 locs x 16 batch = 128 cols]
    moving     = 8 locations' weight slices [K, 8 x 64 outch = 512 cols]
    psum       = [128, 512] f32 = exactly one 2 KB PSUM bank
  Only the 8 diagonal [16, 64] blocks of each product are real output (7/8
  of the MACs are discarded -- PE time is N cycles, so this is free).
  Engines require 32-aligned partition bases, so the drain copies pull
  32-partition x 128-col blocks (the real [16, 64] block plus its
  neighbor's garbage) as fp16, alternating DVE/ACT; the host drops the
  garbage halves.

  Per location the 9 taps x 64 channels contract as 5 accumulating matmuls:
    - chunks 0-2 (K=128): tap pairs {0,3},{1,4},{2,5} stacked on the
      partition dim; the upper 64 partitions hold x shifted by one padded
      row (58) so one AP reads both taps.
    - chunk 3 (K=128): taps {6,7}, upper half shifted by 1.
    - chunk 4 (K=64): tap 8 on one partition half; even rows use the lower
      half (plain x), odd rows the upper half (shift-1 x) with the slot's
      other half carrying the adjacent row's tap-8 weights, so slots stay
      dense. Row 6 (unpaired) wastes its upper halves.
  x is staged as [128, region, f, b] (batch innermost) so the 128 stationary
  columns of each matmul are one strided AP.
"""

import numpy as np

B, C, O, H, W = 16, 64, 64, 56, 56
NCORES = 8
HPC = H // NCORES          # 7 output rows per core
XROWS = HPC + 2            # 9 padded-x rows per core
XW = W + 2                 # 58
BLK = XROWS * XW           # 522 x columns per (c)
XTRIM = BLK - 2 * XW       # 406 cols actually addressed per region
XOFF = 2 * XW              # region-0 window starts at padded row 2
GRP = 8                    # output locations per matmul
NG = W // GRP              # 7 groups per row
# weight slots: [g][c0 x8][c1 x8][c2 x8][c3 x8][(c4 x8 if h even)]
NS_EVEN = NG * 5 * GRP     # 280 slots (rows 0,2,4,6: carry tap8 for h,h+1)
NS_ODD = NG * 4 * GRP      # 224 slots
SLAB_OFF = [0, 280, 504, 784, 1008, 1288, 1512]
NS_TOT = 1792
WSCALE = 32.0              # w stored as e3m4*32, x as fp16/32

_CACHE = {}


def _host_prep(x, weight):
    """Build per-core device input arrays (layout transforms, host-side only)."""
    x = np.ascontiguousarray(x, dtype=np.float32)
    w0 = weight.reshape(O, C, H, W, 9).astype(np.float32, copy=False)

    xpad = np.zeros((B, C, H + 2, W + 2), np.float32)
    xpad[:, :, 1:-1, 1:-1] = x

    xs_list, ws_list = [], []
    for core in range(NCORES):
        h0 = core * HPC
        # x: [128, region, f, b]; region 0 = [plain; shift-1],
        # region 1 = [plain; shift-58]; b innermost.
        xc = xpad[:, :, h0:h0 + XROWS, :]                     # [B, C, 9, 58]
        plain = np.ascontiguousarray(
            xc.transpose(1, 2, 3, 0)).reshape(C, BLK, B)      # [C, f, b]
        sh1 = np.zeros_like(plain)
        sh1[:, :-1] = plain[:, 1:]
        sh58 = np.zeros_like(plain)
        sh58[:, :-58] = plain[:, 58:]
        xdev = np.empty((128, 2, XTRIM, B), np.float32)
        xdev[:64, 0] = plain[:, XOFF:]
        xdev[64:, 0] = sh1[:, XOFF:]
        xdev[:64, 1] = plain[:, :XTRIM]
        xdev[64:, 1] = sh58[:, :XTRIM]
        xs_list.append((xdev / WSCALE).astype(np.float16))

        # weight slots: [128, NS_TOT, 64]
        wc = w0[:, :, h0:h0 + HPC, :, :]                      # [O, C, 7, 56, 9]
        wt = np.ascontiguousarray(wc.transpose(1, 4, 2, 3, 0))  # [C, 9, 7, 56, O]
        S = np.zeros((128, NS_TOT, O), np.float32)
        for h in range(HPC):
            off = SLAB_OFF[h]
            bs = 5 * GRP if h % 2 == 0 else 4 * GRP
            for g in range(NG):
                base = off + g * bs
                w_lo = g * GRP
                for t in range(3):
                    S[:64, base + t * GRP: base + (t + 1) * GRP] = \
                        wt[:, t, h, w_lo:w_lo + GRP]
                    S[64:, base + t * GRP: base + (t + 1) * GRP] = \
                        wt[:, t + 3, h, w_lo:w_lo + GRP]
                S[:64, base + 24: base + 32] = wt[:, 6, h, w_lo:w_lo + GRP]
                S[64:, base + 24: base + 32] = wt[:, 7, h, w_lo:w_lo + GRP]
                if h % 2 == 0:
                    S[:64, base + 32: base + 40] = wt[:, 8, h, w_lo:w_lo + GRP]
                    if h + 1 < HPC:
                        S[64:, base + 32: base + 40] = \
                            wt[:, 8, h + 1, w_lo:w_lo + GRP]
        import ml_dtypes
        ws_list.append((S * WSCALE).astype(ml_dtypes.float8_e3m4))
    return xs_list, ws_list


def _build_program(repeat=1, mode="full"):
    import concourse.mybir as mybir
    import concourse.tile as tile
    from concourse import bacc

    f32 = mybir.dt.float32
    f16 = mybir.dt.float16
    f8 = mybir.dt.float8e3
    nc = bacc.Bacc("TRN2", target_bir_lowering=False, debug=False,
                   num_devices=NCORES)
    xs = nc.dram_tensor("xs", [128, 2, XTRIM, B], f16,
                        kind="ExternalInput")
    ws = nc.dram_tensor("ws", [128, NS_TOT, O], f8, kind="ExternalInput")
    out = nc.dram_tensor("out", [HPC, 128, NG, 2, O], f16,
                         kind="ExternalOutput")

    with tile.TileContext(nc) as tc:
        with tc.tile_pool(name="xp",
                          bufs=(2 if mode in ("full_u2", "full_2dma")
                                else 4)) as xpool, \
             tc.tile_pool(name="xq", bufs=1) as xqpool, \
             tc.tile_pool(name="wp", bufs=4) as wpool, \
             tc.tile_pool(name="op", bufs=3) as opool, \
             tc.tile_pool(name="pq",
                          bufs=(3 if mode == "full_p3" else 4),
                          space="PSUM") as pqpool, \
             tc.tile_pool(name="pp", bufs=1, space="PSUM") as ppool:

            xt_pre = None
            if mode.endswith("noxs") or mode == "pe_xs2":
                xt_pre = xqpool.tile([128, 2, XTRIM, B], f16, name="xtp")
                nc.scalar.dma_start(xt_pre[:], xs[:])
            wt_pre = None
            if mode == "full_wthru":
                wt_pre = xqpool.tile([128, NS_EVEN, O], f8, name="wtp")
                nc.sync.dma_start(wt_pre[:], ws[:, 0:NS_EVEN, :])

            def body(_iv=None):
                # Queue split: weight slabs own the SP queue (pure stream);
                # xs rides ACT, out rides GPSIMD -- a DMA start that waits on
                # compute must not sit ahead of weight prefetches in the
                # strict-FIFO SP queue.
                if xt_pre is not None:
                    xt = xt_pre
                    if mode == "pe_xs2":
                        jt = xpool.tile([128, 2, XTRIM, B], f16, name="xt")
                        nc.scalar.dma_start(jt[:, :, 0:3 * XW, :],
                                            xs[:, :, 0:3 * XW, :])
                        nc.scalar.dma_start(jt[:, :, 3 * XW:, :],
                                            xs[:, :, 3 * XW:, :])
                else:
                    xt = xpool.tile([128, 2, XTRIM, B], f16, name="xt")
                    if mode != "full_2dma":
                        for c0, c1 in ((0, 102), (102, 204), (204, 306),
                                       (306, XTRIM)):
                            nc.scalar.dma_start(xt[:, :, c0:c1, :],
                                                xs[:, :, c0:c1, :])
                    else:
                        nc.scalar.dma_start(xt[:, :, 0:3 * XW, :],
                                            xs[:, :, 0:3 * XW, :])
                        nc.scalar.dma_start(xt[:, :, 3 * XW:, :],
                                            xs[:, :, 3 * XW:, :])
                wt0 = None
                if mode.startswith("pe"):
                    wt0 = wpool.tile([128, NS_EVEN, O], f8, name="wt")
                    nc.sync.dma_start(wt0[:], ws[:, 0:NS_EVEN, :])
                ncopy = 0
                wt_prev = None
                for h in range(HPC):
                    ns = NS_EVEN if h % 2 == 0 else NS_ODD
                    bs = 5 * GRP if h % 2 == 0 else 4 * GRP
                    if mode.startswith("pe"):
                        wt = wt0
                    else:
                        wt = wpool.tile([128, ns, O], f8, name="wt")
                        o0 = SLAB_OFF[h]
                        nc.sync.dma_start(
                            wt[:, 0:ns // 2, :], ws[:, o0:o0 + ns // 2, :])
                        nc.sync.dma_start(
                            wt[:, ns // 2:, :], ws[:, o0 + ns // 2:o0 + ns, :])
                        if wt_pre is not None:
                            wt = wt_pre
                    if mode == "dma":
                        wt_prev = wt
                        continue
                    ot = opool.tile([128, NG, 2, O], f16, name="ot")
                    pq = None
                    for g in range(NG):
                        base = g * bs
                        F = h * XW + g * GRP
                        G = h * XW + g * GRP
                        if g % 2 == 0:
                            pq = pqpool.tile([128, 2, GRP, O], f32,
                                             name="pq")
                        ps = pq[:, g % 2]
                        for t in range(3):
                            nc.tensor.matmul(
                                ps[:], xt[:, 1, F + t:F + t + GRP, :],
                                wt[:, base + t * GRP:base + (t + 1) * GRP, :],
                                start=(t == 0), stop=False)
                        nc.tensor.matmul(
                            ps[:], xt[:, 0, G:G + GRP, :],
                            wt[:, base + 24:base + 32, :],
                            start=False, stop=(mode == "pe4"))
                        if mode == "pe4":
                            continue
                        if h % 2 == 0 or mode.startswith("pe"):
                            c4 = wt if not mode.startswith("pe") else wt0
                            nc.tensor.matmul(
                                ps[:], xt[0:64, 0, G + 2:G + 2 + GRP, :],
                                c4[0:64, base + 32:base + 40, :],
                                start=False, stop=True)
                        else:
                            pbase = g * 5 * GRP
                            nc.tensor.matmul(
                                ps[:], xt[64:128, 0, G + 1:G + 1 + GRP, :],
                                wt_prev[64:128, pbase + 32:pbase + 40, :],
                                start=False, stop=True)
                        if mode in ("pe_nodrain", "pe_noxs", "pe_xs2"):
                            continue
                        if g % 2 == 1 or g == NG - 1:
                            g0, np_ = (g - 1, 2) if g % 2 == 1 else (g, 1)
                            for j in range(GRP // 2):
                                dst = ot[32 * j:32 * (j + 1),
                                         g0:g0 + np_, :, :]
                                src = pq[32 * j:32 * (j + 1), 0:np_,
                                         2 * j:2 * j + 2, :]
                                if ncopy % 2 == 0:
                                    nc.vector.tensor_copy(dst, src)
                                else:
                                    nc.scalar.copy(dst, src)
                                ncopy += 1
                    if mode in ("pe_nodrain", "pe4", "pe_noxs", "pe_xs2"):
                        continue
                    odma = {"full_ogp": nc.gpsimd.dma_start,
                            "full_osp": nc.sync.dma_start}.get(
                                mode, nc.scalar.dma_start)
                    if h == HPC - 1:
                        # split the last row's out-DMA so the first half
                        # fires before the final drain copies (shorter tail)
                        odma(out[h, 0:64], ot[0:64])
                        odma(out[h, 64:128], ot[64:128])
                    else:
                        odma(out[h], ot[:])
                    wt_prev = wt

            if repeat > 1:
                # hardware loop re-executes the same instructions, so tiles
                # can't rotate across iterations; unroll 2 bodies per trip so
                # the next body's xs DMA overlaps the current body's compute.
                nun = 2 if mode in ("full_u2", "full_2dma") else 4
                assert repeat % nun == 0
                with tc.For_i(0, repeat // nun, 1):
                    for _ in range(nun):
                        body()
            else:
                body()
    nc.compile()
    return nc


def _get_program(repeat=1, mode="full"):
    key = ("nc", repeat, mode)
    if key not in _CACHE:
        _CACHE[key] = _build_program(repeat, mode)
    return _CACHE[key]


def run(x, weight, trace=False, repeat=1, mode="full"):
    from concourse.bass_utils import run_bass_kernel_spmd

    nc = _get_program(repeat, mode)
    xs_list, ws_list = _host_prep(np.asarray(x), np.asarray(weight))
    in_maps = [{"xs": xs_list[i], "ws": ws_list[i]} for i in range(NCORES)]
    res = run_bass_kernel_spmd(nc, in_maps, core_ids=list(range(NCORES)),
                               trace=trace)
    parts = []
    for i in range(NCORES):
        oc = np.asarray(res.results[i]["out"])       # [HPC, 128, NG, 2, O]
        oc = oc.reshape(HPC, 4, 2, B, NG, 2, O)      # [h, j, i2, b, g, e, o]
        real = np.stack([oc[:, :, 0, :, :, 0], oc[:, :, 1, :, :, 1]],
                        axis=2)                      # [h, j, i2, b, g, o]
        parts.append(real.transpose(3, 5, 0, 4, 1, 2)
                     .astype(np.float32).reshape(B, O, HPC, W))
    full = np.concatenate(parts, axis=2)             # [B, O, H, W]
    return np.ascontiguousarray(full), res


def kernel(x, weight):
    out, _ = run(x, weight, trace=False)
    return out



# revision 41
# speedup vs baseline: 1.1619x; 1.0604x over previous
"""LocallyConnected2d (3x3, pad 1) Trainium2 kernel.

Problem: out[b,o,h,w] = sum_{c,k} x_pad[b,c,h+k//3,w+k%3] * W[o,c,h,w,k]
  x: [16, 64, 56, 56] f32, W: [1, 64, 64, 56, 56, 9] f32 -> out [16, 64, 56, 56] f32

Strategy (8 cores, H sharded, 7 output rows per core):
  The weight (462 MB) is used exactly once -> the kernel is bound by
  streaming it HBM->SBUF->PE. W is quantized to fp8 e3m4 (x32 scale; 4
  mantissa bits -> rel err ~1.3e-2 vs the 2e-2 gate) and x to fp16 (/32, so
  the product is exact); the PE runs the mixed fp16-stationary x fp8-moving
  matmul natively, and PSUM accumulates in f32.

  Schedule notes (measured on the axon trn2 cores):
  - DMA writes into SBUF steal ~half their duration from the PE stream
    (write-port contention), so bytes moved matter more than queue layout:
    x ships as 4 column pieces (4-deep body unroll gives each piece a full
    body of prefetch slack), and only the 406 addressed columns of each
    x region are sent.
  - Weight slabs own the strict-FIFO SP queue; x pieces and the output
    ride the ACT queue (HWDGE dispatch is ~3x cheaper than SWDGE on
    gpsimd); drains alternate DVE/ACT 50/50.
  - PSUM: pairs of banks per tile (4 bufs = all 8 banks); a pair of
    location-groups drains with 4 copies of [32, 2, 2, 64].

  PE shape: 8 output locations are packed into every matmul so the moving
  operand is wide (N=512) and the per-matmul fixed costs amortize:
    stationary = x patches [K, 8 locs x 16 batch = 128 cols]
    moving     = 8 locations' weight slices [K, 8 x 64 outch = 512 cols]
    psum       = [128, 512] f32 = exactly one 2 KB PSUM bank
  Only the 8 diagonal [16, 64] blocks of each product are real output (7/8
  of the MACs are discarded -- PE time is N cycles, so this is free).
  Engines require 32-aligned partition bases, so the drain copies pull
  32-partition x 128-col blocks (the real [16, 64] block plus its
  neighbor's garbage) as fp16, alternating DVE/ACT; the host drops the
  garbage halves.

  Per location the 9 taps x 64 channels contract as 5 accumulating matmuls:
    - chunks 0-2 (K=128): tap pairs {0,3},{1,4},{2,5} stacked on the
      partition dim; the upper 64 partitions hold x shifted by one padded
      row (58) so one AP reads both taps.
    - chunk 3 (K=128): taps {6,7}, upper half shifted by 1.
    - chunk 4 (K=64): tap 8 on one partition half; even rows use the lower
      half (plain x), odd rows the upper half (shift-1 x) with the slot's
      other half carrying the adjacent row's tap-8 weights, so slots stay
      dense. Row 6 (unpaired) wastes its upper halves.
  x is staged as [128, region, f, b] (batch innermost) so the 128 stationary
  columns of each matmul are one strided AP.
"""

import numpy as np

B, C, O, H, W = 16, 64, 64, 56, 56
NCORES = 8
HPC = H // NCORES          # 7 output rows per core
XROWS = HPC + 2            # 9 padded-x rows per core
XW = W + 2                 # 58
BLK = XROWS * XW           # 522 x columns per (c)
XTRIM = BLK - 2 * XW       # 406 cols actually addressed per region
XOFF = 2 * XW              # region-0 window starts at padded row 2
GRP = 8                    # output locations per matmul
NG = W // GRP              # 7 groups per row
# weight slots: [g][c0 x8][c1 x8][c2 x8][c3 x8][(c4 x8 if h even)]
NS_EVEN = NG * 5 * GRP     # 280 slots (rows 0,2,4,6: carry tap8 for h,h+1)
NS_ODD = NG * 4 * GRP      # 224 slots
SLAB_OFF = [0, 280, 504, 784, 1008, 1288, 1512]
NS_TOT = 1792
WSCALE = 32.0              # w stored as e3m4*32, x as fp16/32

_CACHE = {}


def _host_prep(x, weight):
    """Build per-core device input arrays (layout transforms, host-side only)."""
    x = np.ascontiguousarray(x, dtype=np.float32)
    w0 = weight.reshape(O, C, H, W, 9).astype(np.float32, copy=False)

    xpad = np.zeros((B, C, H + 2, W + 2), np.float32)
    xpad[:, :, 1:-1, 1:-1] = x

    xs_list, ws_list = [], []
    for core in range(NCORES):
        h0 = core * HPC
        # x: [128, region, f, b]; region 0 = [plain; shift-1],
        # region 1 = [plain; shift-58]; b innermost.
        xc = xpad[:, :, h0:h0 + XROWS, :]                     # [B, C, 9, 58]
        plain = np.ascontiguousarray(
            xc.transpose(1, 2, 3, 0)).reshape(C, BLK, B)      # [C, f, b]
        sh1 = np.zeros_like(plain)
        sh1[:, :-1] = plain[:, 1:]
        sh58 = np.zeros_like(plain)
        sh58[:, :-58] = plain[:, 58:]
        xdev = np.empty((128, 2, XTRIM, B), np.float32)
        xdev[:64, 0] = plain[:, XOFF:]
        xdev[64:, 0] = sh1[:, XOFF:]
        xdev[:64, 1] = plain[:, :XTRIM]
        xdev[64:, 1] = sh58[:, :XTRIM]
        xs_list.append((xdev / WSCALE).astype(np.float16))

        # weight slots: [128, NS_TOT, 64]
        wc = w0[:, :, h0:h0 + HPC, :, :]                      # [O, C, 7, 56, 9]
        wt = np.ascontiguousarray(wc.transpose(1, 4, 2, 3, 0))  # [C, 9, 7, 56, O]
        S = np.zeros((128, NS_TOT, O), np.float32)
        for h in range(HPC):
            off = SLAB_OFF[h]
            bs = 5 * GRP if h % 2 == 0 else 4 * GRP
            for g in range(NG):
                base = off + g * bs
                w_lo = g * GRP
                for t in range(3):
                    S[:64, base + t * GRP: base + (t + 1) * GRP] = \
                        wt[:, t, h, w_lo:w_lo + GRP]
                    S[64:, base + t * GRP: base + (t + 1) * GRP] = \
                        wt[:, t + 3, h, w_lo:w_lo + GRP]
                S[:64, base + 24: base + 32] = wt[:, 6, h, w_lo:w_lo + GRP]
                S[64:, base + 24: base + 32] = wt[:, 7, h, w_lo:w_lo + GRP]
                if h % 2 == 0:
                    S[:64, base + 32: base + 40] = wt[:, 8, h, w_lo:w_lo + GRP]
                    if h + 1 < HPC:
                        S[64:, base + 32: base + 40] = \
                            wt[:, 8, h + 1, w_lo:w_lo + GRP]
        import ml_dtypes
        ws_list.append((S * WSCALE).astype(ml_dtypes.float8_e3m4))
    return xs_list, ws_list


def _build_program(repeat=1, mode="full"):
    import concourse.mybir as mybir
    import concourse.tile as tile
    from concourse import bacc

    f32 = mybir.dt.float32
    f16 = mybir.dt.float16
    f8 = mybir.dt.float8e3
    nc = bacc.Bacc("TRN2", target_bir_lowering=False, debug=False,
                   num_devices=NCORES)
    xs = nc.dram_tensor("xs", [128, 2, XTRIM, B], f16,
                        kind="ExternalInput")
    ws = nc.dram_tensor("ws", [128, NS_TOT, O], f8, kind="ExternalInput")
    out = nc.dram_tensor("out", [HPC, 128, NG, 2, O], f16,
                         kind="ExternalOutput")

    with tile.TileContext(nc) as tc:
        with tc.tile_pool(name="xp",
                          bufs=(2 if mode in ("full_u2", "full_2dma")
                                else 4)) as xpool, \
             tc.tile_pool(name="xq", bufs=1) as xqpool, \
             tc.tile_pool(name="wp",
                          bufs=(5 if mode == "full_w5" else 4)) as wpool, \
             tc.tile_pool(name="op", bufs=3) as opool, \
             tc.tile_pool(name="pq",
                          bufs=(3 if mode == "full_p3" else 4),
                          space="PSUM") as pqpool, \
             tc.tile_pool(name="pp", bufs=1, space="PSUM") as ppool:

            xt_pre = None
            if mode.endswith("noxs") or mode == "pe_xs2":
                xt_pre = xqpool.tile([128, 2, XTRIM, B], f16, name="xtp")
                nc.scalar.dma_start(xt_pre[:], xs[:])
            wt_pre = None
            if mode == "full_wthru":
                wt_pre = xqpool.tile([128, NS_EVEN, O], f8, name="wtp")
                nc.sync.dma_start(wt_pre[:], ws[:, 0:NS_EVEN, :])

            def body(_iv=None):
                # Queue split: weight slabs own the SP queue (pure stream);
                # xs rides ACT, out rides GPSIMD -- a DMA start that waits on
                # compute must not sit ahead of weight prefetches in the
                # strict-FIFO SP queue.
                if xt_pre is not None:
                    xt = xt_pre
                    if mode == "pe_xs2":
                        jt = xpool.tile([128, 2, XTRIM, B], f16, name="xt")
                        nc.scalar.dma_start(jt[:, :, 0:3 * XW, :],
                                            xs[:, :, 0:3 * XW, :])
                        nc.scalar.dma_start(jt[:, :, 3 * XW:, :],
                                            xs[:, :, 3 * XW:, :])
                else:
                    xt = xpool.tile([128, 2, XTRIM, B], f16, name="xt")
                    if mode == "full_x8":
                        for p in range(8):
                            c0 = XTRIM * p // 8
                            c1 = XTRIM * (p + 1) // 8
                            nc.scalar.dma_start(xt[:, :, c0:c1, :],
                                                xs[:, :, c0:c1, :])
                    elif mode != "full_2dma":
                        for c0, c1 in ((0, 102), (102, 204), (204, 306),
                                       (306, XTRIM)):
                            nc.scalar.dma_start(xt[:, :, c0:c1, :],
                                                xs[:, :, c0:c1, :])
                    else:
                        nc.scalar.dma_start(xt[:, :, 0:3 * XW, :],
                                            xs[:, :, 0:3 * XW, :])
                        nc.scalar.dma_start(xt[:, :, 3 * XW:, :],
                                            xs[:, :, 3 * XW:, :])
                wt0 = None
                if mode.startswith("pe"):
                    wt0 = wpool.tile([128, NS_EVEN, O], f8, name="wt")
                    nc.sync.dma_start(wt0[:], ws[:, 0:NS_EVEN, :])
                ncopy = 0
                wt_prev = None
                for h in range(HPC):
                    ns = NS_EVEN if h % 2 == 0 else NS_ODD
                    bs = 5 * GRP if h % 2 == 0 else 4 * GRP
                    if mode.startswith("pe"):
                        wt = wt0
                    else:
                        wt = wpool.tile([128, ns, O], f8, name="wt")
                        o0 = SLAB_OFF[h]
                        npc = 4 if mode == "full_w4" else 2
                        for p in range(npc):
                            a = ns * p // npc
                            b2 = ns * (p + 1) // npc
                            nc.sync.dma_start(
                                wt[:, a:b2, :], ws[:, o0 + a:o0 + b2, :])
                        if wt_pre is not None:
                            wt = wt_pre
                    if mode == "dma":
                        wt_prev = wt
                        continue
                    ot = opool.tile([128, NG, 2, O], f16, name="ot")
                    pq = None
                    for g in range(NG):
                        base = g * bs
                        F = h * XW + g * GRP
                        G = h * XW + g * GRP
                        if g % 2 == 0:
                            pq = pqpool.tile([128, 2, GRP, O], f32,
                                             name="pq")
                        ps = pq[:, g % 2]
                        for t in range(3):
                            nc.tensor.matmul(
                                ps[:], xt[:, 1, F + t:F + t + GRP, :],
                                wt[:, base + t * GRP:base + (t + 1) * GRP, :],
                                start=(t == 0), stop=False)
                        nc.tensor.matmul(
                            ps[:], xt[:, 0, G:G + GRP, :],
                            wt[:, base + 24:base + 32, :],
                            start=False, stop=(mode == "pe4"))
                        if mode == "pe4":
                            continue
                        if h % 2 == 0 or mode.startswith("pe"):
                            c4 = wt if not mode.startswith("pe") else wt0
                            nc.tensor.matmul(
                                ps[:], xt[0:64, 0, G + 2:G + 2 + GRP, :],
                                c4[0:64, base + 32:base + 40, :],
                                start=False, stop=True)
                        else:
                            pbase = g * 5 * GRP
                            nc.tensor.matmul(
                                ps[:], xt[64:128, 0, G + 1:G + 1 + GRP, :],
                                wt_prev[64:128, pbase + 32:pbase + 40, :],
                                start=False, stop=True)
                        if mode in ("pe_nodrain", "pe_noxs", "pe_xs2"):
                            continue
                        if g % 2 == 1 or g == NG - 1:
                            g0, np_ = (g - 1, 2) if g % 2 == 1 else (g, 1)
                            for j in range(GRP // 2):
                                dst = ot[32 * j:32 * (j + 1),
                                         g0:g0 + np_, :, :]
                                src = pq[32 * j:32 * (j + 1), 0:np_,
                                         2 * j:2 * j + 2, :]
                                if mode == "full_d23":
                                    use_dve = ncopy % 3 != 2
                                else:
                                    use_dve = ncopy % 2 == 0
                                if use_dve:
                                    nc.vector.tensor_copy(dst, src)
                                else:
                                    nc.scalar.copy(dst, src)
                                ncopy += 1
                    if mode in ("pe_nodrain", "pe4", "pe_noxs", "pe_xs2"):
                        continue
                    odma = {"full_ogp": nc.gpsimd.dma_start,
                            "full_osp": nc.sync.dma_start}.get(
                                mode, nc.scalar.dma_start)
                    if h == HPC - 1:
                        # split the last row's out-DMA so the first half
                        # fires before the final drain copies (shorter tail)
                        odma(out[h, 0:64], ot[0:64])
                        odma(out[h, 64:128], ot[64:128])
                    else:
                        odma(out[h], ot[:])
                    wt_prev = wt

            if repeat > 1:
                # hardware loop re-executes the same instructions, so tiles
                # can't rotate across iterations; unroll 2 bodies per trip so
                # the next body's xs DMA overlaps the current body's compute.
                nun = 2 if mode in ("full_u2", "full_2dma") else 4
                while repeat % nun:
                    nun //= 2
                with tc.For_i(0, repeat // nun, 1):
                    for _ in range(nun):
                        body()
            else:
                body()
    nc.compile()
    return nc


def _get_program(repeat=1, mode="full"):
    key = ("nc", repeat, mode)
    if key not in _CACHE:
        _CACHE[key] = _build_program(repeat, mode)
    return _CACHE[key]


def run(x, weight, trace=False, repeat=1, mode="full"):
    from concourse.bass_utils import run_bass_kernel_spmd

    nc = _get_program(repeat, mode)
    xs_list, ws_list = _host_prep(np.asarray(x), np.asarray(weight))
    in_maps = [{"xs": xs_list[i], "ws": ws_list[i]} for i in range(NCORES)]
    res = run_bass_kernel_spmd(nc, in_maps, core_ids=list(range(NCORES)),
                               trace=trace)
    parts = []
    for i in range(NCORES):
        oc = np.asarray(res.results[i]["out"])       # [HPC, 128, NG, 2, O]
        oc = oc.reshape(HPC, 4, 2, B, NG, 2, O)      # [h, j, i2, b, g, e, o]
        real = np.stack([oc[:, :, 0, :, :, 0], oc[:, :, 1, :, :, 1]],
                        axis=2)                      # [h, j, i2, b, g, o]
        parts.append(real.transpose(3, 5, 0, 4, 1, 2)
                     .astype(np.float32).reshape(B, O, HPC, W))
    full = np.concatenate(parts, axis=2)             # [B, O, H, W]
    return np.ascontiguousarray(full), res


def kernel(x, weight):
    out, _ = run(x, weight, trace=False)
    return out

